# revision 2
# baseline (speedup 1.0000x reference)
"""Multi-head attention (B=2, S=2048, D=1024, H=16, d_k=64) on 8 TRN2 NeuronCores.

V2: I/O-minimal variant. The measured per-execution cost is dominated by
host<->device staging of the NEFF's I/O tensors, so V2 ships every element
exactly once, in fp16, and uses in-NEFF collectives to fan data out/in:

  - Core c: batch b=c//4, head group g=c%4 (4 heads), S-shard r=c%4.
  - Inputs per core: transposed fp16 S-shards xqT_s/xkT_s/xvT_s = X[b][512r:
    512(r+1), :].T (each [1024, 512], 1MB), fp16 weight slices (Wq/Wk/Wv
    columns, Wo rows), f32 q/k biases, fp16 v bias.
  - In-NEFF AllGather (groups [[0..3],[4..7]]) concatenates the 4 rank
    blocks into [4096, 512]: block q4 is exactly X^T[:, 512*q4:512*(q4+1)],
    i.e. the per-quarter X^T layout the projection loops consume directly.
  - Attention per head group as in V1 (fp16 operands, f32 PSUM): scores via
    2-head-packed matmuls, one exp ACT per [128,1024] tile, [v|1]^T E
    accumulation with softmax denominator in row 64, reciprocal+broadcast
    normalize, output projection pipelined one quarter behind.
  - Partial outputs (this head group's Wo rows contribution, fp16) go to a
    DRAM buffer; ReduceScatter(add) over the batch group leaves each core
    its final [512, 1024] slice, shipped back fp16. Host adds bo in f32.

Error budget: fp16 input rounding ~5e-4 + fp16 matmul/exp path ~1e-3,
against a 2e-2 gate.
"""

import numpy as np

import concourse.bacc as bacc
import concourse.mybir as mybir
import concourse.tile as tile
from concourse.bass_utils import run_bass_kernel_spmd

dt = mybir.dt

S = 2048
D = 1024
DH = 256  # head dims per core (4 heads x 64)
DK = 64
P = 128
NK = D // P  # 8 contraction chunks for projections
NM = DH // P  # 2 row groups of qT/kT
NST = S // P  # 16 sk tiles
NQ4 = S // 512  # 4 sq quarters
SS = S // 4  # 512 rows per S-shard
NCORES = 8
VW = 65  # v columns per head incl. ones column
GROUPS = [[0, 1, 2, 3], [4, 5, 6, 7]]

F16 = dt.float16
F32 = dt.float32


def _build_program(reps=1):
    nc = bacc.Bacc("TRN2", target_bir_lowering=False, debug=False,
                   num_devices=NCORES)

    xqT_s = nc.dram_tensor("xqT_s", [D, SS], F16, kind="ExternalInput").ap()
    xkT_s = nc.dram_tensor("xkT_s", [D, SS], F16, kind="ExternalInput").ap()
    xvT_s = nc.dram_tensor("xvT_s", [D, SS], F16, kind="ExternalInput").ap()
    wq = nc.dram_tensor("wq", [D, DH], F16, kind="ExternalInput").ap()
    wk = nc.dram_tensor("wk", [D, DH], F16, kind="ExternalInput").ap()
    wv = nc.dram_tensor("wv", [D, DH], F16, kind="ExternalInput").ap()
    wo = nc.dram_tensor("wo", [DH, D], F16, kind="ExternalInput").ap()
    bq = nc.dram_tensor("bq", [DH, 1], F32, kind="ExternalInput").ap()
    bk = nc.dram_tensor("bk", [DH, 1], F32, kind="ExternalInput").ap()
    bv = nc.dram_tensor("bv", [1, DH], F16, kind="ExternalInput").ap()
    onesd = nc.dram_tensor("onesd", [1, P], F16, kind="ExternalInput").ap()
    vones = nc.dram_tensor("vones", [P, NST * 4], F16, kind="ExternalInput").ap()
    y = nc.dram_tensor("y", [SS, D], F16, kind="ExternalOutput").ap()

    with tile.TileContext(nc) as tc:
        with tc.tile_pool(name="dram", bufs=1, space="DRAM") as dram, \
             tc.tile_pool(name="persist", bufs=1) as pp_sb, \
             tc.tile_pool(name="xq_pool", bufs=12) as xq_pool, \
             tc.tile_pool(name="xv_pool", bufs=10) as xv_pool, \
             tc.tile_pool(name="e_pool", bufs=4) as e_pool, \
             tc.tile_pool(name="nrm_pool", bufs=4) as nrm_pool, \
             tc.tile_pool(name="y_pool", bufs=3) as y_pool:

            # ---- DRAM staging: shard bounces, gathered X^T, partial/final y
            xqT_b = dram.tile([D, SS], F16, tag="xqT_b")
            xkT_b = dram.tile([D, SS], F16, tag="xkT_b")
            xvT_b = dram.tile([D, SS], F16, tag="xvT_b")
            xqT_g = dram.tile([NQ4 * D, SS], F16, tag="xqT_g")
            xkT_g = dram.tile([NQ4 * D, SS], F16, tag="xkT_g")
            xvT_g = dram.tile([NQ4 * D, SS], F16, tag="xvT_g")
            yb = dram.tile([S, D], F16, tag="yb")
            ybr = dram.tile([SS, D], F16, tag="ybr")

            # ---- persistent SBUF ----
            wq_sb = pp_sb.tile([P, NK, DH], F16, tag="wq_sb")
            wk_sb = pp_sb.tile([P, NK, DH], F16, tag="wk_sb")
            wv_sb = pp_sb.tile([P, NK, DH], F16, tag="wv_sb")
            wo_sb = pp_sb.tile([P, NM, D], F16, tag="wo_sb")
            bq_sb = pp_sb.tile([P, NM], F32, tag="bq_sb")
            bk_sb = pp_sb.tile([P, NM], F32, tag="bk_sb")
            bv_sb = pp_sb.tile([1, DH], F16, tag="bv_sb")
            ones_sb = pp_sb.tile([1, P], F16, tag="ones_sb")
            qT_sb = pp_sb.tile([P, NM, S], F16, tag="qT_sb")
            kT_sb = pp_sb.tile([P, NM, S], F16, tag="kT_sb")
            v_sb = pp_sb.tile([P, NST, 4 * VW], F16, tag="v_sb")
            otn_sb = pp_sb.tile([P, NM, S], F16, tag="otn_sb")

            # input shard -> bounce -> all-gather (kv first: kv stage starts)
            nc.sync.dma_start(out=xkT_b, in_=xkT_s)
            nc.sync.dma_start(out=xvT_b, in_=xvT_s)
            nc.sync.dma_start(out=xqT_b, in_=xqT_s)
            nc.gpsimd.collective_compute(
                "AllGather", mybir.AluOpType.bypass, replica_groups=GROUPS,
                ins=[xkT_b[:].opt()], outs=[xkT_g[:].opt()])
            nc.gpsimd.collective_compute(
                "AllGather", mybir.AluOpType.bypass, replica_groups=GROUPS,
                ins=[xvT_b[:].opt()], outs=[xvT_g[:].opt()])
            nc.gpsimd.collective_compute(
                "AllGather", mybir.AluOpType.bypass, replica_groups=GROUPS,
                ins=[xqT_b[:].opt()], outs=[xqT_g[:].opt()])

            nc.sync.dma_start(out=ones_sb, in_=onesd)
            nc.sync.dma_start(out=bk_sb, in_=bk.rearrange("(m p) o -> p (m o)", p=P))
            nc.sync.dma_start(out=bv_sb, in_=bv)
            nc.sync.dma_start(out=wk_sb, in_=wk.rearrange("(k p) n -> p k n", p=P))
            nc.sync.dma_start(out=wv_sb, in_=wv.rearrange("(k p) n -> p k n", p=P))
            nc.sync.dma_start(out=wq_sb, in_=wq.rearrange("(k p) n -> p k n", p=P))
            # ones columns of v (col 64 of each head block)
            v_ones_ap = v_sb[:].rearrange("p s (h x) -> p s h x", x=VW)[:, :, :, DK:DK + 1]
            nc.sync.dma_start(
                out=v_ones_ap,
                in_=vones.rearrange("p (s h o) -> p s h o", s=NST, h=4))
            nc.sync.dma_start(out=bq_sb, in_=bq.rearrange("(m p) o -> p (m o)", p=P))
            nc.sync.dma_start(out=wo_sb, in_=wo.rearrange("(c p) n -> p c n", p=P))
            # warm the exp table set so the first real exp doesn't pay the load
            warm_sb = pp_sb.tile([1, P], F32, tag="warm_sb")
            nc.scalar.activation(warm_sb[:], ones_sb[:],
                                 mybir.ActivationFunctionType.Exp)

            for rep in range(reps):
                # ---- stage KV: k and v projections (full S needed by attn)
                with tc.tile_pool(name="psA", bufs=1, space="PSUM") as psA:
                    for n4 in range(NQ4):
                        ppm = [psA.tile([P, 512], F32, tag=f"pp{m}", bufs=2,
                                        name=f"pp{m}")
                               for m in range(NM)]
                        for k in range(NK):
                            xt = xq_pool.tile([P, 512], F16, tag="xt",
                                              name="xt")
                            nc.sync.dma_start(
                                out=xt,
                                in_=xkT_g[n4 * D + k * P:n4 * D + (k + 1) * P, :])
                            for m in range(NM):
                                nc.tensor.matmul(
                                    ppm[m][:], wk_sb[:, k, m * P:(m + 1) * P],
                                    xt[:], start=(k == 0), stop=(k == NK - 1))
                        for m in range(NM):
                            nc.vector.tensor_scalar_add(
                                kT_sb[:, m, n4 * 512:(n4 + 1) * 512],
                                ppm[m][:], bk_sb[:, m:m + 1])

                    for n4 in range(NQ4):
                        pvs = [psA.tile([P, DH], F32, tag=f"pv{i}", bufs=1,
                                        name=f"pv{i}")
                               for i in range(4)]
                        for k in range(NK):
                            xvb = xv_pool.tile([P, 512], F16, tag="xvb",
                                               name="xvb")
                            nc.sync.dma_start(
                                out=xvb,
                                in_=xvT_g[n4 * D + k * P:n4 * D + (k + 1) * P, :])
                            for sti in range(4):
                                nc.tensor.matmul(
                                    pvs[sti][:], xvb[:, sti * P:(sti + 1) * P],
                                    wv_sb[:, k, :], start=(k == 0), stop=False,
                                    skip_group_check=True)
                        for sti in range(4):
                            st = 4 * n4 + sti
                            nc.tensor.matmul(pvs[sti][:], ones_sb[:], bv_sb[:],
                                             start=False, stop=True,
                                             skip_group_check=True)
                            v_dst = v_sb[:, st, :].rearrange(
                                "p (h x) -> p h x", x=VW)[:, :, 0:DK]
                            v_src = pvs[sti][:].rearrange(
                                "p (h x) -> p h x", x=DK)
                            nc.vector.tensor_copy(v_dst, v_src)

                # ---- main loop: per sq quarter: q proj -> attention,
                #      with the output projection pipelined one quarter behind
                with tc.tile_pool(name="psM", bufs=1, space="PSUM") as psM:
                    def emit_out_proj(q4o):
                        for t in range(4 * q4o, 4 * (q4o + 1)):
                            yt = y_pool.tile([P, D], F16, tag="yt", name="yt")
                            for n in range(2):
                                yps = psM.tile([P, 512], F32, tag="stp",
                                               bufs=2, name="yps")
                                for cc in range(NM):
                                    nc.tensor.matmul(
                                        yps[:],
                                        otn_sb[:, cc, t * P:(t + 1) * P],
                                        wo_sb[:, cc, n * 512:(n + 1) * 512],
                                        start=(cc == 0), stop=(cc == NM - 1),
                                        skip_group_check=True)
                                if n == 0:
                                    nc.scalar.copy(
                                        yt[:, n * 512:(n + 1) * 512], yps[:])
                                else:
                                    nc.vector.tensor_copy(
                                        yt[:, n * 512:(n + 1) * 512], yps[:])
                            nc.sync.dma_start(out=yb[t * P:(t + 1) * P, :],
                                              in_=yt[:])

                    for q4 in range(NQ4):
                        qs = slice(q4 * 512, (q4 + 1) * 512)
                        # q projection for this quarter (both m-halves in one
                        # accumulator tile, separate accumulation groups)
                        stq = psM.tile([P, 1024], F32, tag="stp", bufs=2,
                                       name="stq")
                        for k in range(NK):
                            xt = xq_pool.tile([P, 512], F16, tag="xt",
                                              name="xt")
                            nc.sync.dma_start(
                                out=xt,
                                in_=xqT_g[q4 * D + k * P:q4 * D + (k + 1) * P, :])
                            for m in range(NM):
                                nc.tensor.matmul(
                                    stq[:, m * 512:(m + 1) * 512],
                                    wq_sb[:, k, m * P:(m + 1) * P], xt[:],
                                    start=(k == 0), stop=(k == NK - 1),
                                    skip_group_check=True)
                        for m in range(NM):
                            nc.vector.tensor_scalar_add(
                                qT_sb[:, m, qs], stq[:, m * 512:(m + 1) * 512],
                                bq_sb[:, m:m + 1])

                        for c in range(2):  # head pairs (2c, 2c+1)
                            otp = [psM.tile([VW, 512], F32, tag=f"ot{i}",
                                            bufs=2, name=f"ot{i}")
                                   for i in range(2)]
                            for kt in range(NST):
                                stp = psM.tile([P, 1024], F32, tag="stp",
                                               bufs=2, name="stp")
                                for i in range(2):  # head within pair
                                    pa = 64 * i
                                    nc.tensor.matmul(
                                        stp[:, i * 512:(i + 1) * 512],
                                        kT_sb[pa:pa + DK, c,
                                              kt * P:(kt + 1) * P],
                                        qT_sb[pa:pa + DK, c, qs],
                                        start=True, stop=True)
                                et = e_pool.tile([P, 1024], F16, tag="et")
                                nc.scalar.activation(
                                    et[:], stp[:],
                                    mybir.ActivationFunctionType.Exp,
                                    scale=0.125)
                                for i in range(2):
                                    h = 2 * c + i
                                    nc.tensor.matmul(
                                        otp[i][:],
                                        v_sb[:, kt, h * VW:(h + 1) * VW],
                                        et[:, i * 512:(i + 1) * 512],
                                        start=(kt == 0), stop=(kt == NST - 1),
                                        skip_group_check=True)
                            for i in (1, 0):
                                # normalize: rows 0-63 /= row 64
                                rs = nrm_pool.tile([P, 512], F32, tag="rs")
                                nc.vector.reciprocal(rs[DK:DK + 1, :],
                                                     otp[i][DK:DK + 1, :])
                                rs0 = nrm_pool.tile([1, 512], F32, tag="rs0")
                                nc.sync.dma_start(out=rs0,
                                                  in_=rs[DK:DK + 1, :])
                                rb = nrm_pool.tile([DK, 512], F32, tag="rb")
                                nc.gpsimd.partition_broadcast(rb[:], rs0[:])
                                if i == 0:
                                    nc.vector.tensor_mul(otn_sb[0:DK, c, qs],
                                                         otp[i][0:DK, :],
                                                         rb[:])
                                else:
                                    tmp = nrm_pool.tile([DK, 512], F16,
                                                        tag="tmp")
                                    nc.vector.tensor_mul(tmp[:],
                                                         otp[i][0:DK, :],
                                                         rb[:])
                                    nc.sync.dma_start(
                                        out=otn_sb[DK:P, c, qs], in_=tmp[:])

                        if q4 > 0:
                            emit_out_proj(q4 - 1)
                    emit_out_proj(NQ4 - 1)

                # ---- reduce partials across the batch group; core rank r
                #      keeps rows [512r, 512r+512) of the batch's output
                nc.gpsimd.collective_compute(
                    "ReduceScatter", mybir.AluOpType.add, replica_groups=GROUPS,
                    ins=[yb[:].opt()], outs=[ybr[:].opt()])
                nc.sync.dma_start(out=y, in_=ybr[:])

    nc.compile()
    return nc


_NC = None


def _get_program():
    global _NC
    if _NC is None:
        _NC = _build_program()
    return _NC


def _make_in_maps(Q, K, V, Wq, bq, Wk, bk, Wv, bv, Wo):
    Qh = np.asarray(Q, np.float16)
    Kh = np.asarray(K, np.float16)
    Vh = np.asarray(V, np.float16)
    wqs, wks, wvs, wos, in_maps = [], [], [], [], []
    for g in range(4):
        cols = slice(g * DH, (g + 1) * DH)
        wqs.append(np.ascontiguousarray(Wq[:, cols].astype(np.float16)))
        wks.append(np.ascontiguousarray(Wk[:, cols].astype(np.float16)))
        wvs.append(np.ascontiguousarray(Wv[:, cols].astype(np.float16)))
        wos.append(np.ascontiguousarray(Wo[cols, :].astype(np.float16)))
    onesd = np.ones((1, P), np.float16)
    vones = np.ones((P, NST * 4), np.float16)
    for c in range(NCORES):
        b = c // 4
        g = c % 4
        rows = slice(g * SS, (g + 1) * SS)
        cols = slice(g * DH, (g + 1) * DH)
        in_maps.append({
            "xqT_s": np.ascontiguousarray(Qh[b, rows].T),
            "xkT_s": np.ascontiguousarray(Kh[b, rows].T),
            "xvT_s": np.ascontiguousarray(Vh[b, rows].T),
            "wq": wqs[g],
            "wk": wks[g],
            "wv": wvs[g],
            "wo": wos[g],
            "bq": np.ascontiguousarray(bq[cols].astype(np.float32).reshape(DH, 1)),
            "bk": np.ascontiguousarray(bk[cols].astype(np.float32).reshape(DH, 1)),
            "bv": np.ascontiguousarray(bv[cols].astype(np.float16).reshape(1, DH)),
            "onesd": onesd,
            "vones": vones,
        })
    return in_maps


def run(inputs, trace=False):
    """Returns (full_output [2, S, D] float32, exec_time_ns or None)."""
    nc = _get_program()
    in_maps = _make_in_maps(
        np.asarray(inputs["Q"], np.float32), np.asarray(inputs["K"], np.float32),
        np.asarray(inputs["V"], np.float32), np.asarray(inputs["Wq"], np.float32),
        np.asarray(inputs["bq"], np.float32), np.asarray(inputs["Wk"], np.float32),
        np.asarray(inputs["bk"], np.float32), np.asarray(inputs["Wv"], np.float32),
        np.asarray(inputs["bv"], np.float32), np.asarray(inputs["Wo"], np.float32))
    res = run_bass_kernel_spmd(nc, in_maps, core_ids=list(range(NCORES)),
                               trace=trace)
    bo = np.asarray(inputs["bo"], np.float32)
    out = np.empty((2, S, D), np.float32)
    for c in range(NCORES):
        b = c // 4
        r = c % 4
        out[b, r * SS:(r + 1) * SS] = res.results[c]["y"].astype(np.float32) + bo
    return out, res.exec_time_ns


def kernel(**inputs):
    out, _ = run(inputs, trace=False)
    return out


# revision 8
# speedup vs baseline: 1.0155x; 1.0155x over previous
"""Multi-head attention (B=2, S=2048, D=1024, H=16, d_k=64) on 8 TRN2 NeuronCores.

V3: I/O-minimal variant. The measured per-execution cost is dominated by
host<->device staging of the NEFF's I/O tensors, so V3 ships every element
exactly once, in fp16, and uses in-NEFF collectives to fan data out/in:

  - Core c: batch b=c//4, head group g=c%4 (4 heads), S-shard r=c%4.
  - Inputs per core: transposed fp16 S-shards xqT_s/xkT_s/xvT_s = X[b][512r:
    512(r+1), :].T (each [1024, 512], 1MB), HALF of each fp16 weight slice
    (Wq/Wk/Wv columns, Wo rows; the pair {c, c+4} holds the same slice and
    a 2-core pair AllGather reconstructs it), f32 q/k biases, fp16 v bias.
  - In-NEFF AllGather (groups [[0..3],[4..7]]) concatenates the 4 rank
    blocks into [4096, 512]: block q4 is exactly X^T[:, 512*q4:512*(q4+1)],
    i.e. the per-quarter X^T layout the projection loops consume directly.
  - Attention per head group as in V1 (fp16 operands, f32 PSUM): scores via
    2-head-packed matmuls, one exp ACT per [128,1024] tile, [v|1]^T E
    accumulation with softmax denominator in row 64, reciprocal+broadcast
    normalize, output projection pipelined one quarter behind.
  - Partial outputs (this head group's Wo rows contribution, fp16) go to a
    DRAM buffer; ReduceScatter(add) over the batch group leaves each core
    its final [512, 1024] slice, shipped back fp16. Host adds bo in f32.

Error budget: fp16 input rounding ~5e-4 + fp16 matmul/exp path ~1e-3,
against a 2e-2 gate.
"""

import numpy as np

import concourse.bacc as bacc
import concourse.mybir as mybir
import concourse.tile as tile
from concourse.bass_utils import run_bass_kernel_spmd

dt = mybir.dt

S = 2048
D = 1024
DH = 256  # head dims per core (4 heads x 64)
DK = 64
P = 128
NK = D // P  # 8 contraction chunks for projections
NM = DH // P  # 2 row groups of qT/kT
NST = S // P  # 16 sk tiles
NQ4 = S // 512  # 4 sq quarters
SS = S // 4  # 512 rows per S-shard
NCORES = 8
VW = 65  # v columns per head incl. ones column
GROUPS = [[0, 1, 2, 3], [4, 5, 6, 7]]  # batch groups (X gather, y reduce)
PGROUPS = [[0, 4], [1, 5], [2, 6], [3, 7]]  # same-head-slice pairs (W gather)

F16 = dt.float16
F32 = dt.float32


def _build_program(reps=1):
    nc = bacc.Bacc("TRN2", target_bir_lowering=False, debug=False,
                   num_devices=NCORES)

    xqT_s = nc.dram_tensor("xqT_s", [D, SS], F16, kind="ExternalInput").ap()
    xkT_s = nc.dram_tensor("xkT_s", [D, SS], F16, kind="ExternalInput").ap()
    xvT_s = nc.dram_tensor("xvT_s", [D, SS], F16, kind="ExternalInput").ap()
    # each core ships only half its weight slices; 2-core pair AllGathers
    # ([c, c+4] hold the same head-group slice) reconstruct the full slices
    wq_h = nc.dram_tensor("wq_h", [D // 2, DH], F16, kind="ExternalInput").ap()
    wk_h = nc.dram_tensor("wk_h", [D // 2, DH], F16, kind="ExternalInput").ap()
    wv_h = nc.dram_tensor("wv_h", [D // 2, DH], F16, kind="ExternalInput").ap()
    wo_h = nc.dram_tensor("wo_h", [DH // 2, D], F16, kind="ExternalInput").ap()
    bq = nc.dram_tensor("bq", [DH, 1], F32, kind="ExternalInput").ap()
    bk = nc.dram_tensor("bk", [DH, 1], F32, kind="ExternalInput").ap()
    bv = nc.dram_tensor("bv", [1, DH], F16, kind="ExternalInput").ap()
    y = nc.dram_tensor("y", [SS, D], F16, kind="ExternalOutput").ap()

    with tile.TileContext(nc) as tc:
        with tc.tile_pool(name="dram", bufs=1, space="DRAM") as dram, \
             tc.tile_pool(name="persist", bufs=1) as pp_sb, \
             tc.tile_pool(name="xq_pool", bufs=12) as xq_pool, \
             tc.tile_pool(name="xv_pool", bufs=10) as xv_pool, \
             tc.tile_pool(name="e_pool", bufs=4) as e_pool, \
             tc.tile_pool(name="nrm_pool", bufs=4) as nrm_pool, \
             tc.tile_pool(name="y_pool", bufs=3) as y_pool:

            # ---- DRAM staging: shard bounces, gathered X^T, partial/final y
            xqT_b = dram.tile([D, SS], F16, tag="xqT_b")
            xkT_b = dram.tile([D, SS], F16, tag="xkT_b")
            xvT_b = dram.tile([D, SS], F16, tag="xvT_b")
            xqT_g = dram.tile([NQ4 * D, SS], F16, tag="xqT_g")
            xkT_g = dram.tile([NQ4 * D, SS], F16, tag="xkT_g")
            xvT_g = dram.tile([NQ4 * D, SS], F16, tag="xvT_g")
            wq_hb = dram.tile([D // 2, DH], F16, tag="wq_hb")
            wk_hb = dram.tile([D // 2, DH], F16, tag="wk_hb")
            wv_hb = dram.tile([D // 2, DH], F16, tag="wv_hb")
            wo_hb = dram.tile([DH // 2, D], F16, tag="wo_hb")
            wq_g = dram.tile([D, DH], F16, tag="wq_g")
            wk_g = dram.tile([D, DH], F16, tag="wk_g")
            wv_g = dram.tile([D, DH], F16, tag="wv_g")
            wo_g = dram.tile([DH, D], F16, tag="wo_g")
            yb = dram.tile([S, D], F16, tag="yb")
            ybr = dram.tile([SS, D], F16, tag="ybr")

            # ---- persistent SBUF ----
            wq_sb = pp_sb.tile([P, NK, DH], F16, tag="wq_sb")
            wk_sb = pp_sb.tile([P, NK, DH], F16, tag="wk_sb")
            wv_sb = pp_sb.tile([P, NK, DH], F16, tag="wv_sb")
            wo_sb = pp_sb.tile([P, NM, D], F16, tag="wo_sb")
            bq_sb = pp_sb.tile([P, NM], F32, tag="bq_sb")
            bk_sb = pp_sb.tile([P, NM], F32, tag="bk_sb")
            bv_sb = pp_sb.tile([1, DH], F16, tag="bv_sb")
            ones_sb = pp_sb.tile([1, P], F16, tag="ones_sb")
            qT_sb = pp_sb.tile([P, NM, S], F16, tag="qT_sb")
            kT_sb = pp_sb.tile([P, NM, S], F16, tag="kT_sb")
            v_sb = pp_sb.tile([P, NST, 4 * VW], F16, tag="v_sb")
            otn_sb = pp_sb.tile([P, NM, S], F16, tag="otn_sb")

            # input shard / weight half -> bounce -> all-gather; weight pair
            # gathers are tiny and come first so SBUF weight loads can start
            nc.sync.dma_start(out=wk_hb, in_=wk_h)
            nc.sync.dma_start(out=wv_hb, in_=wv_h)
            nc.sync.dma_start(out=xkT_b, in_=xkT_s)
            nc.sync.dma_start(out=xvT_b, in_=xvT_s)
            nc.sync.dma_start(out=wq_hb, in_=wq_h)
            nc.sync.dma_start(out=xqT_b, in_=xqT_s)
            nc.sync.dma_start(out=wo_hb, in_=wo_h)
            nc.gpsimd.collective_compute(
                "AllGather", mybir.AluOpType.bypass, replica_groups=PGROUPS,
                ins=[wk_hb[:].opt()], outs=[wk_g[:].opt()])
            nc.gpsimd.collective_compute(
                "AllGather", mybir.AluOpType.bypass, replica_groups=PGROUPS,
                ins=[wv_hb[:].opt()], outs=[wv_g[:].opt()])
            nc.gpsimd.collective_compute(
                "AllGather", mybir.AluOpType.bypass, replica_groups=GROUPS,
                ins=[xkT_b[:].opt()], outs=[xkT_g[:].opt()])
            nc.gpsimd.collective_compute(
                "AllGather", mybir.AluOpType.bypass, replica_groups=GROUPS,
                ins=[xvT_b[:].opt()], outs=[xvT_g[:].opt()])
            nc.gpsimd.collective_compute(
                "AllGather", mybir.AluOpType.bypass, replica_groups=PGROUPS,
                ins=[wq_hb[:].opt()], outs=[wq_g[:].opt()])
            nc.gpsimd.collective_compute(
                "AllGather", mybir.AluOpType.bypass, replica_groups=GROUPS,
                ins=[xqT_b[:].opt()], outs=[xqT_g[:].opt()])
            nc.gpsimd.collective_compute(
                "AllGather", mybir.AluOpType.bypass, replica_groups=PGROUPS,
                ins=[wo_hb[:].opt()], outs=[wo_g[:].opt()])

            nc.vector.memset(ones_sb[:], 1.0)
            nc.sync.dma_start(out=bk_sb, in_=bk.rearrange("(m p) o -> p (m o)", p=P))
            nc.sync.dma_start(out=bv_sb, in_=bv)
            nc.sync.dma_start(out=wk_sb, in_=wk_g[:].rearrange("(k p) n -> p k n", p=P))
            nc.sync.dma_start(out=wv_sb, in_=wv_g[:].rearrange("(k p) n -> p k n", p=P))
            nc.sync.dma_start(out=wq_sb, in_=wq_g[:].rearrange("(k p) n -> p k n", p=P))
            # ones columns of v (col 64 of each head block)
            v_ones_ap = v_sb[:].rearrange("p s (h x) -> p s h x", x=VW)[:, :, :, DK:DK + 1]
            nc.vector.memset(v_ones_ap, 1.0)
            nc.sync.dma_start(out=bq_sb, in_=bq.rearrange("(m p) o -> p (m o)", p=P))
            nc.sync.dma_start(out=wo_sb, in_=wo_g[:].rearrange("(c p) n -> p c n", p=P))
            # warm the exp table set so the first real exp doesn't pay the load
            warm_sb = pp_sb.tile([1, P], F32, tag="warm_sb")
            nc.scalar.activation(warm_sb[:], ones_sb[:],
                                 mybir.ActivationFunctionType.Exp)

            for rep in range(reps):
                # ---- stage KV: k and v projections (full S needed by attn)
                with tc.tile_pool(name="psA", bufs=1, space="PSUM") as psA:
                    for n4 in range(NQ4):
                        ppm = [psA.tile([P, 512], F32, tag=f"pp{m}", bufs=2,
                                        name=f"pp{m}")
                               for m in range(NM)]
                        for k in range(NK):
                            xt = xq_pool.tile([P, 512], F16, tag="xt",
                                              name="xt")
                            nc.sync.dma_start(
                                out=xt,
                                in_=xkT_g[n4 * D + k * P:n4 * D + (k + 1) * P, :])
                            for m in range(NM):
                                nc.tensor.matmul(
                                    ppm[m][:], wk_sb[:, k, m * P:(m + 1) * P],
                                    xt[:], start=(k == 0), stop=(k == NK - 1))
                        for m in range(NM):
                            nc.vector.tensor_scalar_add(
                                kT_sb[:, m, n4 * 512:(n4 + 1) * 512],
                                ppm[m][:], bk_sb[:, m:m + 1])

                    for n4 in range(NQ4):
                        pvs = [psA.tile([P, DH], F32, tag=f"pv{i}", bufs=1,
                                        name=f"pv{i}")
                               for i in range(4)]
                        for k in range(NK):
                            xvb = xv_pool.tile([P, 512], F16, tag="xvb",
                                               name="xvb")
                            nc.sync.dma_start(
                                out=xvb,
                                in_=xvT_g[n4 * D + k * P:n4 * D + (k + 1) * P, :])
                            for sti in range(4):
                                nc.tensor.matmul(
                                    pvs[sti][:], xvb[:, sti * P:(sti + 1) * P],
                                    wv_sb[:, k, :], start=(k == 0), stop=False,
                                    skip_group_check=True)
                        for sti in range(4):
                            st = 4 * n4 + sti
                            nc.tensor.matmul(pvs[sti][:], ones_sb[:], bv_sb[:],
                                             start=False, stop=True,
                                             skip_group_check=True)
                            v_dst = v_sb[:, st, :].rearrange(
                                "p (h x) -> p h x", x=VW)[:, :, 0:DK]
                            v_src = pvs[sti][:].rearrange(
                                "p (h x) -> p h x", x=DK)
                            nc.vector.tensor_copy(v_dst, v_src)

                # ---- main loop: per sq quarter: q proj -> attention,
                #      with the output projection pipelined one quarter behind
                with tc.tile_pool(name="psM", bufs=1, space="PSUM") as psM:
                    def emit_out_proj(q4o):
                        for t in range(4 * q4o, 4 * (q4o + 1)):
                            yt = y_pool.tile([P, D], F16, tag="yt", name="yt")
                            for n in range(2):
                                yps = psM.tile([P, 512], F32, tag="stp",
                                               bufs=2, name="yps")
                                for cc in range(NM):
                                    nc.tensor.matmul(
                                        yps[:],
                                        otn_sb[:, cc, t * P:(t + 1) * P],
                                        wo_sb[:, cc, n * 512:(n + 1) * 512],
                                        start=(cc == 0), stop=(cc == NM - 1),
                                        skip_group_check=True)
                                if n == 0:
                                    nc.scalar.copy(
                                        yt[:, n * 512:(n + 1) * 512], yps[:])
                                else:
                                    nc.vector.tensor_copy(
                                        yt[:, n * 512:(n + 1) * 512], yps[:])
                            nc.sync.dma_start(out=yb[t * P:(t + 1) * P, :],
                                              in_=yt[:])

                    for q4 in range(NQ4):
                        qs = slice(q4 * 512, (q4 + 1) * 512)
                        # q projection for this quarter (both m-halves in one
                        # accumulator tile, separate accumulation groups)
                        stq = psM.tile([P, 1024], F32, tag="stp", bufs=2,
                                       name="stq")
                        for k in range(NK):
                            xt = xq_pool.tile([P, 512], F16, tag="xt",
                                              name="xt")
                            nc.sync.dma_start(
                                out=xt,
                                in_=xqT_g[q4 * D + k * P:q4 * D + (k + 1) * P, :])
                            for m in range(NM):
                                nc.tensor.matmul(
                                    stq[:, m * 512:(m + 1) * 512],
                                    wq_sb[:, k, m * P:(m + 1) * P], xt[:],
                                    start=(k == 0), stop=(k == NK - 1),
                                    skip_group_check=True)
                        for m in range(NM):
                            nc.vector.tensor_scalar_add(
                                qT_sb[:, m, qs], stq[:, m * 512:(m + 1) * 512],
                                bq_sb[:, m:m + 1])

                        for c in range(2):  # head pairs (2c, 2c+1)
                            otp = [psM.tile([VW, 512], F32, tag=f"ot{i}",
                                            bufs=2, name=f"ot{i}")
                                   for i in range(2)]
                            for kt in range(NST):
                                stp = psM.tile([P, 1024], F32, tag="stp",
                                               bufs=2, name="stp")
                                for i in range(2):  # head within pair
                                    pa = 64 * i
                                    nc.tensor.matmul(
                                        stp[:, i * 512:(i + 1) * 512],
                                        kT_sb[pa:pa + DK, c,
                                              kt * P:(kt + 1) * P],
                                        qT_sb[pa:pa + DK, c, qs],
                                        start=True, stop=True)
                                et = e_pool.tile([P, 1024], F16, tag="et")
                                nc.scalar.activation(
                                    et[:], stp[:],
                                    mybir.ActivationFunctionType.Exp,
                                    scale=0.125)
                                for i in range(2):
                                    h = 2 * c + i
                                    nc.tensor.matmul(
                                        otp[i][:],
                                        v_sb[:, kt, h * VW:(h + 1) * VW],
                                        et[:, i * 512:(i + 1) * 512],
                                        start=(kt == 0), stop=(kt == NST - 1),
                                        skip_group_check=True)
                            for i in (1, 0):
                                # normalize: rows 0-63 /= row 64
                                rs = nrm_pool.tile([P, 512], F32, tag="rs")
                                nc.vector.reciprocal(rs[DK:DK + 1, :],
                                                     otp[i][DK:DK + 1, :])
                                rs0 = nrm_pool.tile([1, 512], F32, tag="rs0")
                                nc.sync.dma_start(out=rs0,
                                                  in_=rs[DK:DK + 1, :])
                                rb = nrm_pool.tile([DK, 512], F32, tag="rb")
                                nc.gpsimd.partition_broadcast(rb[:], rs0[:])
                                if i == 0:
                                    nc.vector.tensor_mul(otn_sb[0:DK, c, qs],
                                                         otp[i][0:DK, :],
                                                         rb[:])
                                else:
                                    tmp = nrm_pool.tile([DK, 512], F16,
                                                        tag="tmp")
                                    nc.vector.tensor_mul(tmp[:],
                                                         otp[i][0:DK, :],
                                                         rb[:])
                                    nc.sync.dma_start(
                                        out=otn_sb[DK:P, c, qs], in_=tmp[:])

                        if q4 > 0:
                            emit_out_proj(q4 - 1)
                    emit_out_proj(NQ4 - 1)

                # ---- reduce partials across the batch group; core rank r
                #      keeps rows [512r, 512r+512) of the batch's output
                nc.gpsimd.collective_compute(
                    "ReduceScatter", mybir.AluOpType.add, replica_groups=GROUPS,
                    ins=[yb[:].opt()], outs=[ybr[:].opt()])
                nc.sync.dma_start(out=y, in_=ybr[:])

    nc.compile()
    return nc


_NC = None


def _get_program():
    global _NC
    if _NC is None:
        _NC = _build_program()
    return _NC


def _make_in_maps(Q, K, V, Wq, bq, Wk, bk, Wv, bv, Wo):
    Qh = np.asarray(Q, np.float16)
    Kh = np.asarray(K, np.float16)
    Vh = np.asarray(V, np.float16)
    in_maps = []
    for c in range(NCORES):
        b = c // 4
        g = c % 4
        rows = slice(g * SS, (g + 1) * SS)
        cols = slice(g * DH, (g + 1) * DH)
        # pair partner c +/- 4 holds the same head-group slice: core in batch
        # group 0 ships the top half of each weight slice, group 1 the bottom
        wrows = slice(b * (D // 2), (b + 1) * (D // 2))
        worows = slice(g * DH + b * (DH // 2), g * DH + (b + 1) * (DH // 2))
        in_maps.append({
            "xqT_s": np.ascontiguousarray(Qh[b, rows].T),
            "xkT_s": np.ascontiguousarray(Kh[b, rows].T),
            "xvT_s": np.ascontiguousarray(Vh[b, rows].T),
            "wq_h": np.ascontiguousarray(Wq[wrows, cols].astype(np.float16)),
            "wk_h": np.ascontiguousarray(Wk[wrows, cols].astype(np.float16)),
            "wv_h": np.ascontiguousarray(Wv[wrows, cols].astype(np.float16)),
            "wo_h": np.ascontiguousarray(Wo[worows, :].astype(np.float16)),
            "bq": np.ascontiguousarray(bq[cols].astype(np.float32).reshape(DH, 1)),
            "bk": np.ascontiguousarray(bk[cols].astype(np.float32).reshape(DH, 1)),
            "bv": np.ascontiguousarray(bv[cols].astype(np.float16).reshape(1, DH)),
        })
    return in_maps


def run(inputs, trace=False):
    """Returns (full_output [2, S, D] float32, exec_time_ns or None)."""
    nc = _get_program()
    in_maps = _make_in_maps(
        np.asarray(inputs["Q"], np.float32), np.asarray(inputs["K"], np.float32),
        np.asarray(inputs["V"], np.float32), np.asarray(inputs["Wq"], np.float32),
        np.asarray(inputs["bq"], np.float32), np.asarray(inputs["Wk"], np.float32),
        np.asarray(inputs["bk"], np.float32), np.asarray(inputs["Wv"], np.float32),
        np.asarray(inputs["bv"], np.float32), np.asarray(inputs["Wo"], np.float32))
    res = run_bass_kernel_spmd(nc, in_maps, core_ids=list(range(NCORES)),
                               trace=trace)
    bo = np.asarray(inputs["bo"], np.float32)
    out = np.empty((2, S, D), np.float32)
    for c in range(NCORES):
        b = c // 4
        r = c % 4
        out[b, r * SS:(r + 1) * SS] = res.results[c]["y"].astype(np.float32) + bo
    return out, res.exec_time_ns


def kernel(**inputs):
    out, _ = run(inputs, trace=False)
    return out
